# revision 1
# baseline (speedup 1.0000x reference)
"""Multi-head attention (B=4, S=2048, E=768, H=8, D=96) on 8 TRN2 NeuronCores.

Sharding: token-split — core c handles batch b=c//2, query-half qh=c%2
(1024 queries), computing K/V over the batch's full 2048 keys (redundantly
across the 2 cores of a batch pair). No collectives.

Device-side dataflow per core (all matmuls in fp32r — full-rate PE mode,
~1.8e-4 relative error):
  - x^T [768, 2048] in SBUF, key order rotated so this core's queries are
    always columns 0:1024 (softmax over keys is order-invariant).
  - Q^T_h [96,1024], K^T_h [96,2048] via projection matmuls
    (lhsT = W columns, rhs = x^T).
  - V for a group of 4 heads at a time, in [token, head-dim] layout with an
    extra all-ones column per head (from an appended ones-row of x^T), so the
    PV matmul also produces the softmax denominator for free.
  - energy^T tiles [128 keys, 1024 queries] = K^T_h-tile.T @ Q^T_h; exp on
    ACT (no max subtraction: |energy| <~ 25, safe in fp32); PV accumulates
    out^T [97, 1024] over 16 key tiles.
  - normalization: row 96 of out^T = sum of exp; recip * 96, partition-
    broadcast (Pool engine), multiply in place, spill to DRAM.
  - final projection: out[tok, :] = sum_h attnout^T_h.T @ Wp[head rows],
    accumulated in PSUM over heads, DMA'd straight to the output.

Biases: bq/bk/bv fold in via an extra x^T ones-row matmul step (only
compiled in when nonzero); bp is added on the host.
"""

import os
import sys

import numpy as np
import ml_dtypes

try:
    import concourse.bass as bass  # noqa: F401
except ImportError:  # pragma: no cover - fallback for odd sys.path setups
    for p in (
        "/root/.axon_site",
        "/root/.axon_site/_ro/trn_rl_repo",
        "/root/.axon_site/_ro/pypackages",
        "/opt/trn_rl_repo",
    ):
        if os.path.isdir(p) and p not in sys.path:
            sys.path.append(p)
    import concourse.bass as bass  # noqa: F401

import concourse.mybir as mybir
from concourse import bacc
from concourse.bass_utils import run_bass_kernel_spmd
from concourse.tile import TileContext

B, S, E, H, D = 4, 2048, 768, 8, 96
NQ = S // 2          # queries per core
NCORES = 8
KT_N = S // 128      # 16 key tiles
VCH = 4 * (D + 1)    # V-group chunk width: 4 heads x (96 dims + ones col)
F32 = mybir.dt.float32
F32R = mybir.dt.float32r
BF16 = mybir.dt.bfloat16
EXP = mybir.ActivationFunctionType.Exp

_CACHE: dict = {}


def _enable_ldw_opt():
    """Rewrite walrus args to re-enable LDWEIGHTS dedup: consecutive
    matmuls that reuse the same stationary weights skip the reload.
    Verified against the reference on every run by test.py."""
    import concourse.bass_utils as _bu

    if getattr(_bu, "_ldw_opt_patched", False):
        return
    _orig = _bu.run_command

    def _patched(argv, **kw):
        argv = ["--enable-ldw-opt=true" if a == "--enable-ldw-opt=false"
                else a for a in argv]
        return _orig(argv, **kw)

    _bu.run_command = _patched
    _bu._ldw_opt_patched = True


if os.environ.get("KERNEL_LDW_OPT", "0") == "1":
    _enable_ldw_opt()

# Filled by the last kernel() call (for test harnesses): exec_time_ns etc.
LAST_RESULT = {}


def _build(has_bias: bool):
    nc = bacc.Bacc("TRN2", target_bir_lowering=False, debug=False,
                   num_devices=NCORES)
    xT_d = nc.dram_tensor("xT", [E + 1, S], F32R, kind="ExternalInput").ap()
    wq_d = nc.dram_tensor("Wq", [E + 1, E], F32R, kind="ExternalInput").ap()
    wk_d = nc.dram_tensor("Wk", [E + 1, E], F32R, kind="ExternalInput").ap()
    wv_d = nc.dram_tensor("Wv", [E + 1, 2 * VCH], F32R, kind="ExternalInput").ap()
    wp_d = nc.dram_tensor("Wp", [H, D, E], F32R, kind="ExternalInput").ap()
    out_d = nc.dram_tensor("out", [NQ, E], F32, kind="ExternalOutput").ap()

    with TileContext(nc) as tc:
        with (
            tc.tile_pool(name="dr", bufs=1, space="DRAM") as drpool,
            tc.tile_pool(name="dn", bufs=1) as dnpool,
            tc.tile_pool(name="mi", bufs=2) as mipool,
        ):
            attn_dram = drpool.tile([D, 6 * NQ], F32)
            den_sb = [dnpool.tile([97, NQ], F32, tag="denA", name="denA"),
                      dnpool.tile([97, NQ], F32, tag="denB", name="denB")]
            nc.gpsimd.memset(den_sb[0][:], 1.0)
            nc.gpsimd.memset(den_sb[1][:], 1.0)

            with (
                tc.tile_pool(name="w", bufs=1) as wpool,
                tc.tile_pool(name="qt", bufs=2) as qtpool,
                tc.tile_pool(name="kt", bufs=2) as ktpool,
                tc.tile_pool(name="vg", bufs=1) as vgpool,
                tc.tile_pool(name="pt", bufs=3) as ptpool,
                tc.tile_pool(name="mm", bufs=4, space="PSUM") as mmps,
                tc.tile_pool(name="pv", bufs=2, space="PSUM") as pvps,
            ):
                # --- resident loads -------------------------------------
                xt = [wpool.tile([128, S], F32R, tag=f"xt{k}", name=f"xt{k}")
                      for k in range(6)]
                # query-half columns first: unblocks Q^T/K^T production ASAP
                for k in range(6):
                    nc.sync.dma_start(xt[k][:, 0:NQ],
                                      xT_d[128 * k:128 * (k + 1), 0:NQ])
                wq, wk, wv = [], [], []
                for nm, dram, lst, width in (
                    ("wq", wq_d, wq, E),
                    ("wk", wk_d, wk, E),
                    ("wv", wv_d, wv, 2 * VCH),
                ):
                    for k in range(6):
                        t = wpool.tile([128, width], F32R, tag=f"{nm}{k}")
                        nc.sync.dma_start(t[:], dram[128 * k:128 * (k + 1), :])
                        lst.append(t)
                    if nm == "wk":
                        for k in range(6):
                            nc.sync.dma_start(
                                xt[k][:, NQ:S],
                                xT_d[128 * k:128 * (k + 1), NQ:S])
                ones = wpool.tile([1, S], F32R, tag="ones")
                nc.sync.dma_start(ones[:], xT_d[E:E + 1, :])
                wvb = wpool.tile([1, 2 * VCH], F32R, tag="wvb")
                nc.sync.dma_start(wvb[:], wv_d[E:E + 1, :])
                if has_bias:
                    wqb = wpool.tile([1, E], F32R, tag="wqb")
                    nc.sync.dma_start(wqb[:], wq_d[E:E + 1, :])
                    wkb = wpool.tile([1, E], F32R, tag="wkb")
                    nc.sync.dma_start(wkb[:], wk_d[E:E + 1, :])

                # (weight tile, x^T tile) pairs per contraction step
                q_steps = [(wq[k], xt[k]) for k in range(6)]
                k_steps = [(wk[k], xt[k]) for k in range(6)]
                v_steps = [(wv[k], xt[k]) for k in range(6)]
                if has_bias:
                    q_steps.append((wqb, ones))
                    k_steps.append((wkb, ones))
                    v_steps.append((wvb, ones))

                vg = None
                stg_keep = {}
                for h in range(H):
                    g, j = divmod(h, 4)
                    c0, c1 = D * h, D * h + D

                    # --- Q^T_h [96, 1024] ------------------------------
                    QT = qtpool.tile([D, NQ], F32R, tag="qt")
                    qps = [mmps.tile([D, 512], F32, tag="mm", name=f"q{h}{qc}")
                           for qc in range(2)]
                    for s, (wt, xs) in enumerate(q_steps):
                        for qc in range(2):
                            nc.tensor.matmul(
                                qps[qc][:], (wt[:, c0:c1]),
                                (xs[:, 512 * qc:512 * qc + 512]),
                                start=(s == 0), stop=(s == len(q_steps) - 1))
                    for qc in range(2):
                        nc.vector.tensor_copy(QT[:, 512 * qc:512 * qc + 512],
                                              qps[qc][:])

                    # --- K^T_h [96, 2048] ------------------------------
                    KT = ktpool.tile([D, S], F32R, tag="kt")
                    for half in range(2):
                        kps = [mmps.tile([D, 512], F32, tag="mm",
                                         name=f"k{h}{half}{i}")
                               for i in range(2)]
                        for s, (wt, xs) in enumerate(k_steps):
                            for i in range(2):
                                kc = 2 * half + i
                                nc.tensor.matmul(
                                    kps[i][:], (wt[:, c0:c1]),
                                    (xs[:, 512 * kc:512 * kc + 512]),
                                    start=(s == 0),
                                    stop=(s == len(k_steps) - 1))
                        for i in range(2):
                            kc = 2 * half + i
                            nc.vector.tensor_copy(
                                KT[:, 512 * kc:512 * kc + 512], kps[i][:])

                    # --- V for heads 4g..4g+3, [tok, 4*(96+1)] ---------
                    if j == 0:
                        vg = vgpool.tile([128, KT_N * VCH], F32R, tag="vg")
                        onesf = mipool.tile([128, KT_N * 4], F32, tag="onesf")
                        nc.vector.memset(onesf[:], 1.0)
                        nc.vector.tensor_copy(
                            vg[:].rearrange("p (i j c) -> p i j c",
                                            j=4, c=D + 1)[:, :, :, D],
                            onesf[:].rearrange("p (i j) -> p i j", j=4))
                        for t in range(KT_N):
                            ps = mmps.tile([128, VCH], F32, tag="mm")
                            for s, (wt, xs) in enumerate(v_steps):
                                nc.tensor.matmul(
                                    ps[:], (xs[:, 128 * t:128 * (t + 1)]),
                                    (wt[:, VCH * g:VCH * (g + 1)]),
                                    start=(s == 0), stop=(s == len(v_steps) - 1))
                            nc.vector.tensor_copy(
                                vg[:, VCH * t:VCH * t + VCH].rearrange(
                                    "p (j c) -> p j c", c=D + 1)[:, :, 0:D],
                                ps[:].rearrange(
                                    "p (j c) -> p j c", c=D + 1)[:, :, 0:D])

                    # --- attention for head h --------------------------
                    pvc = pvps.tile([D + 1, NQ], F32, tag="pv",
                                    name=f"pv{h}")
                    voff = 97 * j

                    def emit_pv(i, pT):
                        for qc in range(2):
                            nc.tensor.matmul(
                                pvc[:, 512 * qc:512 * qc + 512],
                                (vg[:, VCH * i + voff:VCH * i + voff + D + 1]),
                                (pT[:, 512 * qc:512 * qc + 512]),
                                start=(i == 0), stop=(i == KT_N - 1))

                    prev = None
                    for i in range(KT_N):
                        pT = ptpool.tile([128, NQ], F32R, tag="pt")
                        for qc in range(2):
                            ps = mmps.tile([128, 512], F32, tag="mm",
                                           name=f"e{h}_{i}_{qc}")
                            nc.tensor.matmul(
                                ps[:], (KT[:, 128 * i:128 * (i + 1)]),
                                (QT[:, 512 * qc:512 * qc + 512]),
                                start=True, stop=True)
                            nc.scalar.activation(
                                pT[:, 512 * qc:512 * qc + 512], ps[:], EXP)
                        if prev is not None:
                            emit_pv(*prev)
                        prev = (i, pT)
                    emit_pv(*prev)

                    # --- normalize (x96 / rowsum) and spill ------------
                    stg = mipool.tile([D, NQ], F32, tag="stg",
                                      name=f"stg{h}")
                    dt_, dp = den_sb[h // 4], 32 * (h % 4)
                    nc.vector.tensor_copy(dt_[dp:dp + 1, :], pvc[D:D + 1, :])
                    nc.vector.tensor_copy(stg[:], pvc[0:D, :])
                    if h < 6:
                        nc.sync.dma_start(attn_dram[:, NQ * h:NQ * (h + 1)],
                                          stg[:])
                    else:
                        stg_keep[h] = stg

            # --- final projection: out = attnout @ Wp (+bp on host) ----
            with (
                tc.tile_pool(name="fw", bufs=1) as fwpool,
                tc.tile_pool(name="fs", bufs=3) as fspool,
                tc.tile_pool(name="fm", bufs=4, space="PSUM") as fmps,
            ):
                rcp = [dnpool.tile([97, NQ], F32, tag="rcpA", name="rcpA"),
                       dnpool.tile([97, NQ], F32, tag="rcpB", name="rcpB")]
                nc.vector.reciprocal(rcp[0][:], den_sb[0][:])
                nc.vector.reciprocal(rcp[1][:], den_sb[1][:])
                wp_t, at_n = [], []
                for h in range(H):
                    wt = fwpool.tile([D, E], F32R, tag=f"wp{h}")
                    nc.sync.dma_start(wt[:], wp_d[h])
                    wp_t.append(wt)
                    if h < 6:
                        at = fspool.tile([D, NQ], F32, tag="at",
                                         name=f"at{h}")
                        nc.sync.dma_start(at[:],
                                          attn_dram[:, NQ * h:NQ * (h + 1)])
                    else:
                        at = stg_keep[h]
                    rt, rp = rcp[h // 4], 32 * (h % 4)
                    tmp = fspool.tile([1, NQ], F32, tag="rtmp",
                                      name=f"rtmp{h}")
                    nc.vector.tensor_copy(tmp[:], rt[rp:rp + 1, :])
                    bc = fspool.tile([D, NQ], F32, tag="bc", name=f"bc{h}")
                    nc.gpsimd.partition_broadcast(bc[:], tmp[:])
                    an = fwpool.tile([D, NQ], F32R, tag=f"an{h}")
                    nc.vector.tensor_mul(an[:], at[:], bc[:])
                    at_n.append(an)
                CHUNKS = ((0, 512), (512, 256))
                for t in range(NQ // 128):
                    fps = fmps.tile([128, E], F32, tag="fm", name=f"f{t}")
                    for h in range(H):
                        for cs, cw in CHUNKS:
                            nc.tensor.matmul(
                                fps[:, cs:cs + cw],
                                (at_n[h][:, 128 * t:128 * (t + 1)]),
                                (wp_t[h][:, cs:cs + cw]),
                                start=(h == 0), stop=(h == H - 1))
                    fo = fspool.tile([128, E], F32, tag="fo", name=f"fo{t}")
                    nc.scalar.copy(fo[:], fps[:])
                    nc.sync.dma_start(
                        out_d[128 * t:128 * (t + 1), :], fo[:])

    nc.compile()
    return nc


def _prep_inputs(x, Wq, bq, Wk, bk, Wv, bv, Wp):
    """Host-side shard prep. Returns (has_bias, per-core in_maps)."""
    has_bias = bool(np.any(bq) or np.any(bk) or np.any(bv))
    wq_aug = np.ascontiguousarray(np.vstack([Wq, bq[None, :]]), dtype=np.float32)
    wk_aug = np.ascontiguousarray(np.vstack([Wk, bk[None, :]]), dtype=np.float32)
    wv_grp = np.zeros((E + 1, 2 * VCH), dtype=np.float32)
    for h in range(H):
        g, j = divmod(h, 4)
        base = VCH * g + 97 * j
        wv_grp[:E, base:base + D] = Wv[:, D * h:D * h + D]
        wv_grp[E, base:base + D] = bv[D * h:D * h + D]
        wv_grp[E, base + D] = 1.0  # ones column (selects x ones-row)
    wp_r = np.ascontiguousarray(Wp.reshape(H, D, E) * float(D),
                            dtype=np.float32)

    in_maps = []
    for c in range(NCORES):
        b, qh = divmod(c, 2)
        xb = x[b]
        if qh == 0:
            xc = xb
        else:
            xc = np.concatenate([xb[NQ:], xb[:NQ]], axis=0)
        xT = np.empty((E + 1, S), dtype=np.float32)
        xT[:E] = xc.T
        xT[E] = 1.0
        in_maps.append({"xT": xT, "Wq": wq_aug, "Wk": wk_aug,
                       "Wv": wv_grp, "Wp": wp_r})
    return has_bias, in_maps


def kernel(x, Wq, bq, Wk, bk, Wv, bv, Wp, bp):
    x = np.asarray(x, dtype=np.float32)
    Wq = np.asarray(Wq, dtype=np.float32)
    bq = np.asarray(bq, dtype=np.float32)
    Wk = np.asarray(Wk, dtype=np.float32)
    bk = np.asarray(bk, dtype=np.float32)
    Wv = np.asarray(Wv, dtype=np.float32)
    bv = np.asarray(bv, dtype=np.float32)
    Wp = np.asarray(Wp, dtype=np.float32)
    bp = np.asarray(bp, dtype=np.float32)
    assert x.shape == (B, S, E), x.shape

    has_bias, in_maps = _prep_inputs(x, Wq, bq, Wk, bk, Wv, bv, Wp)

    if has_bias not in _CACHE:
        _CACHE[has_bias] = _build(has_bias)
    nc = _CACHE[has_bias]

    trace = bool(os.environ.get("BASS_TRACE"))
    if trace and "antenv.axon_hooks" not in sys.modules:
        _register_ntff_shim()
    res = run_bass_kernel_spmd(nc, in_maps, list(range(NCORES)), trace=trace)

    LAST_RESULT.clear()
    LAST_RESULT.update(
        exec_time_ns=res.exec_time_ns,
        mean_exec_time_ns=res.mean_exec_time_ns,
        instructions_and_trace=res.instructions_and_trace,
        profile_json=res.profile_json,
    )

    out = np.empty((B, S, E), dtype=np.float32)
    for c in range(NCORES):
        b, qh = divmod(c, 2)
        out[b, qh * NQ:(qh + 1) * NQ] = res.results[c]["out"]
    if np.any(bp):
        out += bp[None, None, :]
    return out


def _register_ntff_shim():
    """Make run_bass_kernel_spmd's NTFF profiling work in containers that
    lack antenv.axon_hooks (profiles via ctypes into libaxon_pjrt.so)."""
    import contextlib
    import ctypes
    import types

    so = "/opt/axon/libaxon_pjrt.so"
    if not os.path.exists(so):
        return
    lib = ctypes.CDLL(so)
    if not hasattr(lib, "axon_start_nrt_profile"):
        return
    lib.axon_start_nrt_profile.argtypes = [ctypes.POINTER(ctypes.c_int64),
                                           ctypes.c_size_t]
    lib.axon_start_nrt_profile.restype = ctypes.c_int64
    lib.axon_stop_nrt_profile.argtypes = [ctypes.c_char_p]
    lib.axon_stop_nrt_profile.restype = ctypes.c_int64

    @contextlib.contextmanager
    def _hook(output_dir, device_ids):
        import jax

        jax.devices()
        if device_ids:
            ids = (ctypes.c_int64 * len(device_ids))(*device_ids)
            rc = lib.axon_start_nrt_profile(ids, len(device_ids))
        else:
            rc = lib.axon_start_nrt_profile(None, 0)
        if rc != 0:
            raise RuntimeError(f"axon_start_nrt_profile rc={rc}")
        try:
            yield
        finally:
            n = lib.axon_stop_nrt_profile(str(output_dir).encode())
            print(f"ntff profile: {n} file(s) -> {output_dir}", file=sys.stderr)

    mod = types.ModuleType("antenv.axon_hooks")
    mod.get_axon_ntff_profile_hook = lambda: _hook
    mod.set_axon_ntff_profile_hook = lambda h: None
    sys.modules["antenv.axon_hooks"] = mod



# revision 3
# speedup vs baseline: 1.2522x; 1.2522x over previous
"""Multi-head attention (B=4, S=2048, E=768, H=8, D=96) on 8 TRN2 NeuronCores.

Sharding: token-split — core c handles batch b=c//2, query-half qh=c%2
(1024 queries), computing K/V over the batch's full 2048 keys (redundantly
across the 2 cores of a batch pair). No collectives.

v2 (all-bf16 operands): every matmul operand is bf16 (fp32 PSUM accumulate),
which halves input DMA (~8 MB/core), halves SBUF footprint (attention
outputs stay resident — no DRAM spill), and enables fast-weight-load on PE.
Measured end-to-end rel err ~8e-3 vs the fp32 reference (CPU-sim 8.5e-3),
within the 2e-2 gate.

Device-side dataflow per core:
  - x^T [768(+1), 2048] bf16 in SBUF, key order rotated so this core's
    queries are always columns 0:1024.
  - Per head h: Q^T_h [96,1024] and K^T_h [96,2048] via projection matmuls
    (lhsT = W columns, rhs = x^T), PSUM fp32, cast to bf16 on DVE.
  - V for a group of 4 heads at a time, [token, head-dim] layout with an
    extra all-ones column per head, so the PV matmul also produces the
    softmax denominator for free.
  - energy tiles [128 keys, 1024 queries] into a 2-bank PSUM tile; ONE
    batched exp on ACT (no max subtraction: |energy| <~ 24, safe in fp32)
    writing bf16 pT; PV accumulates out^T [97, 1024] over 16 key tiles.
  - inline normalization per head (hidden under the next head's attention):
    reciprocal_approx_fast of the denominator row, Pool partition-broadcast,
    DVE multiply -> resident bf16 attn tile an_h [96, 1024].
  - final projection: out[tok, :] = sum_h an_h.T @ (Wp[head rows] * 96),
    accumulated in PSUM over heads, ACT copy to SBUF, DMA to the output.

Biases: bq/bk/bv fold in via an extra x^T ones-row matmul step (only
compiled in when nonzero); bp is added on the host.
"""

import os
import sys

import numpy as np
import ml_dtypes

try:
    import concourse.bass as bass  # noqa: F401
except ImportError:  # pragma: no cover - fallback for odd sys.path setups
    for p in (
        "/root/.axon_site",
        "/root/.axon_site/_ro/trn_rl_repo",
        "/root/.axon_site/_ro/pypackages",
        "/opt/trn_rl_repo",
    ):
        if os.path.isdir(p) and p not in sys.path:
            sys.path.append(p)
    import concourse.bass as bass  # noqa: F401

import concourse.mybir as mybir
from concourse import bacc
from concourse.bass_utils import run_bass_kernel_spmd
from concourse.tile import TileContext

B, S, E, H, D = 4, 2048, 768, 8, 96
NQ = S // 2          # queries per core
NCORES = 8
KT_N = S // 128      # 16 key tiles
VCH = 4 * (D + 1)    # V-group chunk width: 4 heads x (96 dims + ones col)
F32 = mybir.dt.float32
BF16 = mybir.dt.bfloat16
EXP = mybir.ActivationFunctionType.Exp

_CACHE: dict = {}


def _enable_ldw_opt():
    """Rewrite walrus args to re-enable LDWEIGHTS dedup: consecutive
    matmuls that reuse the same stationary weights skip the reload.
    Verified against the reference on every run by test.py."""
    import concourse.bass_utils as _bu

    if getattr(_bu, "_ldw_opt_patched", False):
        return
    _orig = _bu.run_command

    def _patched(argv, **kw):
        argv = ["--enable-ldw-opt=true" if a == "--enable-ldw-opt=false"
                else a for a in argv]
        return _orig(argv, **kw)

    _bu.run_command = _patched
    _bu._ldw_opt_patched = True


if os.environ.get("KERNEL_LDW_OPT", "0") == "1":
    _enable_ldw_opt()

# Filled by the last kernel() call (for test harnesses): exec_time_ns etc.
LAST_RESULT = {}


def _build(has_bias: bool):
    nc = bacc.Bacc("TRN2", target_bir_lowering=False, debug=False,
                   num_devices=NCORES)
    xT_d = nc.dram_tensor("xT", [E + 1, S], BF16, kind="ExternalInput").ap()
    wq_d = nc.dram_tensor("Wq", [E + 1, E], BF16, kind="ExternalInput").ap()
    wk_d = nc.dram_tensor("Wk", [E + 1, E], BF16, kind="ExternalInput").ap()
    wv_d = nc.dram_tensor("Wv", [E + 1, 2 * VCH], BF16, kind="ExternalInput").ap()
    wp_d = nc.dram_tensor("Wp", [H, D, E], BF16, kind="ExternalInput").ap()
    out_d = nc.dram_tensor("out", [NQ, E], F32, kind="ExternalOutput").ap()

    with TileContext(nc) as tc:
        with (
            tc.tile_pool(name="w", bufs=1) as wpool,
            tc.tile_pool(name="an", bufs=1) as anpool,
            tc.tile_pool(name="fs", bufs=2) as fspool,
        ):
            # --- resident loads, interleaved for early compute start -----
            xt = [wpool.tile([128, S], BF16, tag=f"xt{k}", name=f"xt{k}")
                  for k in range(6)]
            wq, wk, wv = [], [], []
            for k in range(6):
                t = wpool.tile([128, E], BF16, tag=f"wq{k}")
                nc.sync.dma_start(t[:], wq_d[128 * k:128 * (k + 1), :])
                wq.append(t)
                nc.sync.dma_start(xt[k][:, 0:NQ],
                                  xT_d[128 * k:128 * (k + 1), 0:NQ])
            for k in range(6):
                t = wpool.tile([128, E], BF16, tag=f"wk{k}")
                nc.sync.dma_start(t[:], wk_d[128 * k:128 * (k + 1), :])
                wk.append(t)
                nc.sync.dma_start(xt[k][:, NQ:S],
                                  xT_d[128 * k:128 * (k + 1), NQ:S])
            for k in range(6):
                t = wpool.tile([128, 2 * VCH], BF16, tag=f"wv{k}")
                nc.sync.dma_start(t[:], wv_d[128 * k:128 * (k + 1), :])
                wv.append(t)
            wp_t = []
            for h in range(H):
                t = wpool.tile([D, E], BF16, tag=f"wp{h}")
                nc.sync.dma_start(t[:], wp_d[h])
                wp_t.append(t)
            if has_bias:
                ones = wpool.tile([1, S], BF16, tag="ones")
                nc.sync.dma_start(ones[:], xT_d[E:E + 1, :])
                wvb = wpool.tile([1, 2 * VCH], BF16, tag="wvb")
                nc.sync.dma_start(wvb[:], wv_d[E:E + 1, :])
                wqb = wpool.tile([1, E], BF16, tag="wqb")
                nc.sync.dma_start(wqb[:], wq_d[E:E + 1, :])
                wkb = wpool.tile([1, E], BF16, tag="wkb")
                nc.sync.dma_start(wkb[:], wk_d[E:E + 1, :])

            # (weight tile, x^T tile) pairs per contraction step
            q_steps = [(wq[k], xt[k]) for k in range(6)]
            k_steps = [(wk[k], xt[k]) for k in range(6)]
            v_steps = [(wv[k], xt[k]) for k in range(6)]
            if has_bias:
                q_steps.append((wqb, ones))
                k_steps.append((wkb, ones))
                v_steps.append((wvb, ones))

            an_t = []
            with (
                tc.tile_pool(name="qt", bufs=2) as qtpool,
                tc.tile_pool(name="kt", bufs=2) as ktpool,
                tc.tile_pool(name="vg", bufs=1) as vgpool,
                tc.tile_pool(name="pt", bufs=3) as ptpool,
                tc.tile_pool(name="nm", bufs=3) as nmpool,
                tc.tile_pool(name="pj", bufs=2, space="PSUM") as pjps,
                tc.tile_pool(name="ee", bufs=2, space="PSUM") as eeps,
                tc.tile_pool(name="pv", bufs=1, space="PSUM") as pvps,
            ):
                vg = None
                for h in range(H):
                    g, j = divmod(h, 4)
                    c0, c1 = D * h, D * h + D

                    # --- Q^T_h [96, 1024], bf16 ------------------------
                    QT = qtpool.tile([D, NQ], BF16, tag="qt")
                    qps = [pjps.tile([D, 512], F32, tag="pj", name=f"q{h}{qc}")
                           for qc in range(2)]
                    for s, (wt, xs) in enumerate(q_steps):
                        for qc in range(2):
                            nc.tensor.matmul(
                                qps[qc][:], (wt[:, c0:c1]),
                                (xs[:, 512 * qc:512 * qc + 512]),
                                start=(s == 0), stop=(s == len(q_steps) - 1))
                    for qc in range(2):
                        nc.vector.tensor_copy(QT[:, 512 * qc:512 * qc + 512],
                                              qps[qc][:])

                    # --- K^T_h [96, 2048], bf16 ------------------------
                    KT = ktpool.tile([D, S], BF16, tag="kt")
                    for half in range(2):
                        kps = [pjps.tile([D, 512], F32, tag="pj",
                                         name=f"k{h}{half}{i}")
                               for i in range(2)]
                        for s, (wt, xs) in enumerate(k_steps):
                            for i in range(2):
                                kc = 2 * half + i
                                nc.tensor.matmul(
                                    kps[i][:], (wt[:, c0:c1]),
                                    (xs[:, 512 * kc:512 * kc + 512]),
                                    start=(s == 0),
                                    stop=(s == len(k_steps) - 1))
                        for i in range(2):
                            kc = 2 * half + i
                            nc.vector.tensor_copy(
                                KT[:, 512 * kc:512 * kc + 512], kps[i][:])

                    # --- V for heads 4g..4g+3, [tok, 4*(96+1)] bf16 ----
                    if j == 0:
                        vg = vgpool.tile([128, KT_N * VCH], BF16, tag="vg")
                        onesf = nmpool.tile([128, KT_N * 4], BF16,
                                            tag="onesf")
                        nc.vector.memset(onesf[:], 1.0)
                        nc.vector.tensor_copy(
                            vg[:].rearrange("p (i j c) -> p i j c",
                                            j=4, c=D + 1)[:, :, :, D],
                            onesf[:].rearrange("p (i j) -> p i j", j=4))
                        for t in range(KT_N):
                            ps = pjps.tile([128, VCH], F32, tag="pj")
                            for s, (wt, xs) in enumerate(v_steps):
                                nc.tensor.matmul(
                                    ps[:], (xs[:, 128 * t:128 * (t + 1)]),
                                    (wt[:, VCH * g:VCH * (g + 1)]),
                                    start=(s == 0), stop=(s == len(v_steps) - 1))
                            nc.vector.tensor_copy(
                                vg[:, VCH * t:VCH * t + VCH].rearrange(
                                    "p (j c) -> p j c", c=D + 1)[:, :, 0:D],
                                ps[:].rearrange(
                                    "p (j c) -> p j c", c=D + 1)[:, :, 0:D])

                    # --- attention for head h --------------------------
                    pvc = pvps.tile([D + 1, NQ], F32, tag="pv",
                                    name=f"pv{h}")
                    voff = 97 * j

                    def emit_pv(i, pT):
                        for qc in range(2):
                            nc.tensor.matmul(
                                pvc[:, 512 * qc:512 * qc + 512],
                                (vg[:, VCH * i + voff:VCH * i + voff + D + 1]),
                                (pT[:, 512 * qc:512 * qc + 512]),
                                start=(i == 0), stop=(i == KT_N - 1))

                    prev = None
                    for i in range(KT_N):
                        pT = ptpool.tile([128, NQ], BF16, tag="pt")
                        eps = eeps.tile([128, NQ], F32, tag="ee",
                                        name=f"e{h}_{i}")
                        for qc in range(2):
                            nc.tensor.matmul(
                                eps[:, 512 * qc:512 * qc + 512],
                                (KT[:, 128 * i:128 * (i + 1)]),
                                (QT[:, 512 * qc:512 * qc + 512]),
                                start=True, stop=True)
                        nc.scalar.activation(pT[:], eps[:], EXP)
                        if prev is not None:
                            emit_pv(*prev)
                        prev = (i, pT)
                    emit_pv(*prev)

                    # --- inline normalize -> resident bf16 attn --------
                    an = anpool.tile([D, NQ], BF16, tag=f"an{h}",
                                     name=f"an{h}")
                    den = nmpool.tile([1, NQ], F32, tag="den",
                                      name=f"den{h}")
                    nc.vector.tensor_copy(den[:], pvc[D:D + 1, :])
                    for qc in range(2):
                        s0 = 512 * qc
                        rcp = nmpool.tile([1, 512], F32, tag="rcp",
                                          name=f"rcp{h}{qc}")
                        nc.vector.reciprocal_approx_fast(
                            rcp[:], den[:, s0:s0 + 512])
                        bc = nmpool.tile([D, 512], F32, tag="bc",
                                         name=f"bc{h}{qc}")
                        nc.gpsimd.partition_broadcast(bc[:], rcp[:])
                        nc.vector.tensor_mul(an[:, s0:s0 + 512],
                                             pvc[0:D, s0:s0 + 512], bc[:])
                    an_t.append(an)

            # --- final projection: out = an @ (Wp*96) (+bp on host) ----
            with tc.tile_pool(name="fm", bufs=2, space="PSUM") as fmps:
                CHUNKS = ((0, 512), (512, 256))
                for t in range(NQ // 128):
                    fps = fmps.tile([128, E], F32, tag="fm", name=f"f{t}")
                    for h in range(H):
                        for cs, cw in CHUNKS:
                            nc.tensor.matmul(
                                fps[:, cs:cs + cw],
                                (an_t[h][:, 128 * t:128 * (t + 1)]),
                                (wp_t[h][:, cs:cs + cw]),
                                start=(h == 0), stop=(h == H - 1))
                    fo = fspool.tile([128, E], F32, tag="fo", name=f"fo{t}")
                    nc.scalar.copy(fo[:], fps[:])
                    nc.sync.dma_start(
                        out_d[128 * t:128 * (t + 1), :], fo[:])

    nc.compile()
    return nc


def _prep_inputs(x, Wq, bq, Wk, bk, Wv, bv, Wp):
    """Host-side shard prep (bf16 casts). Returns (has_bias, in_maps)."""
    bf = ml_dtypes.bfloat16
    has_bias = bool(np.any(bq) or np.any(bk) or np.any(bv))
    wq_aug = np.vstack([Wq, bq[None, :]]).astype(bf)
    wk_aug = np.vstack([Wk, bk[None, :]]).astype(bf)
    wv_grp = np.zeros((E + 1, 2 * VCH), dtype=np.float32)
    for h in range(H):
        g, j = divmod(h, 4)
        base = VCH * g + 97 * j
        wv_grp[:E, base:base + D] = Wv[:, D * h:D * h + D]
        wv_grp[E, base:base + D] = bv[D * h:D * h + D]
        wv_grp[E, base + D] = 1.0  # ones column (selects x ones-row)
    wv_grp = wv_grp.astype(bf)
    wp_r = (Wp.reshape(H, D, E) * float(D)).astype(bf)

    in_maps = []
    for c in range(NCORES):
        b, qh = divmod(c, 2)
        xb = x[b]
        if qh == 0:
            xc = xb
        else:
            xc = np.concatenate([xb[NQ:], xb[:NQ]], axis=0)
        xT = np.empty((E + 1, S), dtype=np.float32)
        xT[:E] = xc.T
        xT[E] = 1.0
        in_maps.append({"xT": xT.astype(bf), "Wq": wq_aug, "Wk": wk_aug,
                        "Wv": wv_grp, "Wp": wp_r})
    return has_bias, in_maps


def kernel(x, Wq, bq, Wk, bk, Wv, bv, Wp, bp):
    x = np.asarray(x, dtype=np.float32)
    Wq = np.asarray(Wq, dtype=np.float32)
    bq = np.asarray(bq, dtype=np.float32)
    Wk = np.asarray(Wk, dtype=np.float32)
    bk = np.asarray(bk, dtype=np.float32)
    Wv = np.asarray(Wv, dtype=np.float32)
    bv = np.asarray(bv, dtype=np.float32)
    Wp = np.asarray(Wp, dtype=np.float32)
    bp = np.asarray(bp, dtype=np.float32)
    assert x.shape == (B, S, E), x.shape

    has_bias, in_maps = _prep_inputs(x, Wq, bq, Wk, bk, Wv, bv, Wp)

    if has_bias not in _CACHE:
        _CACHE[has_bias] = _build(has_bias)
    nc = _CACHE[has_bias]

    trace = bool(os.environ.get("BASS_TRACE"))
    if trace and "antenv.axon_hooks" not in sys.modules:
        _register_ntff_shim()
    res = run_bass_kernel_spmd(nc, in_maps, list(range(NCORES)), trace=trace)

    LAST_RESULT.clear()
    LAST_RESULT.update(
        exec_time_ns=res.exec_time_ns,
        mean_exec_time_ns=res.mean_exec_time_ns,
        instructions_and_trace=res.instructions_and_trace,
        profile_json=res.profile_json,
    )

    out = np.empty((B, S, E), dtype=np.float32)
    for c in range(NCORES):
        b, qh = divmod(c, 2)
        out[b, qh * NQ:(qh + 1) * NQ] = res.results[c]["out"]
    if np.any(bp):
        out += bp[None, None, :]
    return out


def _register_ntff_shim():
    """Make run_bass_kernel_spmd's NTFF profiling work in containers that
    lack antenv.axon_hooks (profiles via ctypes into libaxon_pjrt.so)."""
    import contextlib
    import ctypes
    import types

    so = "/opt/axon/libaxon_pjrt.so"
    if not os.path.exists(so):
        return
    lib = ctypes.CDLL(so)
    if not hasattr(lib, "axon_start_nrt_profile"):
        return
    lib.axon_start_nrt_profile.argtypes = [ctypes.POINTER(ctypes.c_int64),
                                           ctypes.c_size_t]
    lib.axon_start_nrt_profile.restype = ctypes.c_int64
    lib.axon_stop_nrt_profile.argtypes = [ctypes.c_char_p]
    lib.axon_stop_nrt_profile.restype = ctypes.c_int64

    @contextlib.contextmanager
    def _hook(output_dir, device_ids):
        import jax

        jax.devices()
        if device_ids:
            ids = (ctypes.c_int64 * len(device_ids))(*device_ids)
            rc = lib.axon_start_nrt_profile(ids, len(device_ids))
        else:
            rc = lib.axon_start_nrt_profile(None, 0)
        if rc != 0:
            raise RuntimeError(f"axon_start_nrt_profile rc={rc}")
        try:
            yield
        finally:
            n = lib.axon_stop_nrt_profile(str(output_dir).encode())
            print(f"ntff profile: {n} file(s) -> {output_dir}", file=sys.stderr)

    mod = types.ModuleType("antenv.axon_hooks")
    mod.get_axon_ntff_profile_hook = lambda: _hook
    mod.set_axon_ntff_profile_hook = lambda h: None
    sys.modules["antenv.axon_hooks"] = mod
